# revision 1
# baseline (speedup 1.0000x reference)
"""Trainium2 Bass kernel for Gemma3 sliding-window attention.

Problem: B=1, T=4096, d_model=2048, 8 query heads / 4 KV heads, head_dim=256,
sliding window 1024, per-head RMSNorm + RoPE (interleaved rotate-half with
cat(freqs,freqs) tables), o_proj.

Sharding (8 cores): 4 KV-head groups x 2 sequence halves. Core (g, s) computes
query heads {2g, 2g+1} and KV head g for query tokens [s*2048, (s+1)*2048),
with a 1024-token KV halo (recomputed locally; s=0's halo is zero-padded and
masked out via the exp bias). Each core emits a partial o-projection
[2048, 2048]; the host sums the 4 group partials per half.

Dataflow: host pre-transposes x and all weights so every matmul operand loads
in its natural layout (contraction on partitions). Projections and attention
matmuls run in bf16 (fp32 PSUM accumulation); the RMSNorm/RoPE chain runs in
fp32/f32r (ssq via ones-matmul broadcast, rotate-half via a constant
permutation matmul). Attention is computed in S.T orientation per 512-query
block: S.T[j,i] = kT.T @ qT per 128-j tile, P.T = exp(S.T/16 + bias) on ACT
(bias -1e5 kills invalid j for the padded half), constant triangle masks on
the 8 window-edge tiles, softmax denominator via ones-matmul (no
max-subtraction: RMSNorm bounds |scores| <= 16), y.T = v.T @ P.T, then y.T is
scaled by the reciprocal denominator and consumed as lhsT by the o-projection.
"""

import sys

if "/opt/trn_rl_repo" not in sys.path:
    sys.path.insert(0, "/opt/trn_rl_repo")

import numpy as np

try:
    import ml_dtypes
    BF16 = ml_dtypes.bfloat16
except ImportError:
    BF16 = None

T, DM, NH, NKV, HD, WIN = 4096, 2048, 8, 4, 256, 1024
EPS, BASE = 1e-6, 10000.0
NG, NS = 4, 2
TL, NQ = 3072, 2048
NTT = 12          # 256-token projection tiles
QT0 = 4           # first query tile
NKO = 16          # 2048 / 128 contraction subtiles
NA = 4            # 512-query attention blocks per core
SCALE = 1.0 / 16.0
NEG = -1.0e5

_cache = {}


def _host_prep(x, pos, Wq, Wk, Wv, Wo, q_norm_w, k_norm_w):
    x = np.asarray(x, np.float32).reshape(T, DM)
    xT = np.ascontiguousarray(x.T)
    pos_f = np.asarray(pos).astype(np.float64)
    m = np.arange(128)
    invf = BASE ** (-m / 128.0)

    Wq = np.asarray(Wq, np.float32)
    Wk = np.asarray(Wk, np.float32)
    Wv = np.asarray(Wv, np.float32)
    Wo = np.asarray(Wo, np.float32)
    qnw = np.asarray(q_norm_w, np.float32)
    knw = np.asarray(k_norm_w, np.float32)

    ones = np.ones((128, 128), np.float32)
    r0T = np.zeros((128, 128), np.float32)
    a = np.arange(64)
    r0T[2 * a, 2 * a + 1] = 1.0
    r0T[2 * a + 1, 2 * a] = -1.0
    qw2 = np.ascontiguousarray(np.stack([qnw[:128], qnw[128:]], axis=1))
    kw2 = np.ascontiguousarray(np.stack([knw[:128], knw[128:]], axis=1))

    # masks for 512-wide attention blocks: m=0..3 far edge, m=8..11 diagonal
    jp = np.arange(128)[:, None]
    ip = np.arange(512)[None, :]
    tris = []
    for mm_ in range(4):
        tris.append(jp >= ip + 1 - 128 * mm_)         # far masks F_m
    for mm_ in range(4):
        tris.append(jp <= ip - 128 * mm_)             # diag masks D_{m+8}
    tri = np.concatenate(tris, axis=1).astype(BF16)   # [128, 8*512]

    in_maps = []
    for g in range(NG):
        for s in range(NS):
            lo = s * 2048 - 1024
            xT_c = np.zeros((DM, TL), np.float32)
            src_lo = max(lo, 0)
            xT_c[:, src_lo - lo:] = xT[:, src_lo:(s + 1) * 2048]
            pidx = np.clip(np.arange(lo, lo + TL), 0, T - 1)
            p = pos_f[pidx]
            p[np.arange(lo, lo + TL) < 0] = 0.0
            ang = p[None, :] * invf[:, None]
            cosk = np.ascontiguousarray(np.cos(ang), dtype=np.float32)
            sink = np.ascontiguousarray(np.sin(ang), dtype=np.float32)

            kbias = np.zeros((128, 24), np.float32)
            if s == 0:
                kbias[:, :8] = NEG

            in_maps.append({
                "xT": xT_c.astype(BF16),
                "cosk": cosk,
                "sink": sink,
                "wqT": np.ascontiguousarray(Wq[2 * g * HD:(2 * g + 2) * HD, :].T).astype(BF16),
                "wkT": np.ascontiguousarray(Wk[g * HD:(g + 1) * HD, :].T).astype(BF16),
                "wvT": np.ascontiguousarray(Wv[g * HD:(g + 1) * HD, :].T).astype(BF16),
                "woT": np.ascontiguousarray(Wo[:, 2 * g * HD:(2 * g + 2) * HD].T).astype(BF16),
                "ones_bf": ones.astype(BF16),
                "r0T": r0T.astype(BF16),
                "qw": qw2,
                "kw": kw2,
                "kbias": kbias,
                "tri": tri,
            })
    return in_maps


def _build_program():
    if "nc" in _cache:
        return _cache["nc"]

    import concourse.bass as bass
    import concourse.mybir as mybir
    import concourse.tile as tile
    from concourse import bacc
    from contextlib import ExitStack

    f32 = mybir.dt.float32
    f32r = mybir.dt.float32r
    bf16 = mybir.dt.bfloat16
    AF = mybir.ActivationFunctionType
    OP = mybir.AluOpType

    nc = bacc.Bacc("TRN2", target_bir_lowering=False, debug=False,
                   enable_asserts=False, num_devices=8)

    xT_d = nc.dram_tensor("xT", [DM, TL], bf16, kind="ExternalInput")
    cosk_d = nc.dram_tensor("cosk", [128, TL], f32, kind="ExternalInput")
    sink_d = nc.dram_tensor("sink", [128, TL], f32, kind="ExternalInput")
    wq_d = nc.dram_tensor("wqT", [DM, 512], bf16, kind="ExternalInput")
    wk_d = nc.dram_tensor("wkT", [DM, 256], bf16, kind="ExternalInput")
    wv_d = nc.dram_tensor("wvT", [DM, 256], bf16, kind="ExternalInput")
    wo_d = nc.dram_tensor("woT", [512, DM], bf16, kind="ExternalInput")
    onesbf_d = nc.dram_tensor("ones_bf", [128, 128], bf16, kind="ExternalInput")
    r0_d = nc.dram_tensor("r0T", [128, 128], bf16, kind="ExternalInput")
    qw_d = nc.dram_tensor("qw", [128, 2], f32, kind="ExternalInput")
    kw_d = nc.dram_tensor("kw", [128, 2], f32, kind="ExternalInput")
    kb_d = nc.dram_tensor("kbias", [128, 24], f32, kind="ExternalInput")
    tri_d = nc.dram_tensor("tri", [128, 8 * 512], bf16, kind="ExternalInput")
    o_d = nc.dram_tensor("o_part", [NQ, DM], f32, kind="ExternalOutput")

    def rv(ap):
        # f32 view of an f32r tile for DVE reads
        return ap.bitcast(f32)

    with tile.TileContext(nc) as tc, ExitStack() as ctx:
        cpool = ctx.enter_context(tc.tile_pool(name="consts", bufs=1))
        xpool = ctx.enter_context(tc.tile_pool(name="xt", bufs=4))
        tabpool = ctx.enter_context(tc.tile_pool(name="tab", bufs=2))
        kpool = ctx.enter_context(tc.tile_pool(name="kring", bufs=5))
        vpool = ctx.enter_context(tc.tile_pool(name="vring", bufs=5))
        scpool = ctx.enter_context(tc.tile_pool(name="scratch", bufs=3))
        spool = ctx.enter_context(tc.tile_pool(name="small", bufs=2))
        qpool = ctx.enter_context(tc.tile_pool(name="qt", bufs=3))
        ptpool = ctx.enter_context(tc.tile_pool(name="pt", bufs=4))
        ypool = ctx.enter_context(tc.tile_pool(name="yt", bufs=3))
        opool = ctx.enter_context(tc.tile_pool(name="osb", bufs=3))
        pp_proj = ctx.enter_context(tc.tile_pool(name="pproj", bufs=3, space="PSUM"))
        pp_small = ctx.enter_context(tc.tile_pool(name="psmall", bufs=2, space="PSUM"))
        pp_acc = ctx.enter_context(tc.tile_pool(name="pacc", bufs=3, space="PSUM"))

        # ---- resident constants / weights ----
        # (order matters: the first tile's k/v projections need wk/wv; wq is
        # needed at tt=4 and wo only at the first attention block)
        wk_sb = cpool.tile([128, NKO, 256], bf16, tag="wk")
        nc.sync.dma_start(wk_sb[:], wk_d.ap().rearrange("(ko p) c -> p ko c", p=128))
        wv_sb = cpool.tile([128, NKO, 256], bf16, tag="wv")
        nc.sync.dma_start(wv_sb[:], wv_d.ap().rearrange("(ko p) c -> p ko c", p=128))
        ones_sb = cpool.tile([128, 128], bf16, tag="ones")
        nc.sync.dma_start(ones_sb[:], onesbf_d.ap())
        onesbf_sb = ones_sb
        r0_sb = cpool.tile([128, 128], bf16, tag="r0")
        nc.sync.dma_start(r0_sb[:], r0_d.ap())
        qw_sb = cpool.tile([128, 2], f32, tag="qwt")
        nc.sync.dma_start(qw_sb[:], qw_d.ap())
        kw_sb = cpool.tile([128, 2], f32, tag="kwt")
        nc.sync.dma_start(kw_sb[:], kw_d.ap())
        kb_sb = cpool.tile([128, 24], f32, tag="kb")
        nc.sync.dma_start(kb_sb[:], kb_d.ap())
        eps_sb = cpool.tile([128, 1], f32, tag="eps")
        nc.vector.memset(eps_sb[:], EPS)
        zero_sb = cpool.tile([128, 1], f32, tag="zero")
        nc.vector.memset(zero_sb[:], 0.0)
        xT_v0 = xT_d.ap().rearrange("(ko p) t -> p ko t", p=128)
        pre_x = []
        for half in range(2):
            xt0 = xpool.tile([128, 8, 512], bf16, tag="xt")
            nc.sync.dma_start(xt0[:], xT_v0[:, half * 8:(half + 1) * 8, 0:512])
            pre_x.append(xt0)
        pre_cos = tabpool.tile([128, 512], f32, tag="cos")
        nc.sync.dma_start(pre_cos[:], cosk_d.ap()[:, 0:512])
        pre_sin = tabpool.tile([128, 512], f32, tag="sin")
        nc.sync.dma_start(pre_sin[:], sink_d.ap()[:, 0:512])
        wq_sb = cpool.tile([128, NKO, 512], bf16, tag="wq")
        nc.sync.dma_start(wq_sb[:], wq_d.ap().rearrange("(ko p) c -> p ko c", p=128))
        tri_sb = cpool.tile([128, 8 * 512], bf16, tag="tri")
        nc.sync.dma_start(tri_sb[:], tri_d.ap())
        wo_sb = cpool.tile([128, 4, DM], bf16, tag="wo")
        nc.sync.dma_start(wo_sb[:], wo_d.ap().rearrange("(hd p) c -> p hd c", p=128))

        xT_v = xT_d.ap().rearrange("(ko p) t -> p ko t", p=128)  # [128, 16, TL]

        NTB = 6                     # 512-token projection tiles
        kt_tiles = [None] * NTB
        vt_tiles = [None] * NTB

        def norm_rope(src_ps, w_sb, cos_t, sin_t, dst, dsti):
            """src_ps: two PSUM [128, 512] tiles (one head's 2 d-subtiles),
            transposed projection over 512 tokens. Writes RMSNorm+RoPE (bf16)
            into dst[:, dsti+u, :]."""
            z2 = scpool.tile([128, 2, 512], bf16, tag="z2")
            for u in range(2):
                nc.scalar.activation(z2[:, u, :], src_ps[u][:], AF.Square,
                                     bias=zero_sb[:])
            ssq = pp_small.tile([128, 512], f32, tag="psm")
            for u in range(2):
                nc.tensor.matmul(ssq[:], ones_sb[:], z2[:, u, :],
                                 start=(u == 0), stop=(u == 1))
            sq = spool.tile([128, 512], f32, tag="sq")
            nc.scalar.activation(sq[:], ssq[:], AF.Sqrt, bias=eps_sb[:], scale=1.0 / HD)
            rs = spool.tile([128, 512], f32, tag="rs")
            nc.vector.reciprocal_approx_fast(rs[:], sq[:])
            znw = scpool.tile([128, 2, 512], bf16, tag="znw")
            t1 = scpool.tile([128, 2, 512], f32, tag="t1")
            for u in range(2):
                nc.vector.scalar_tensor_tensor(
                    znw[:, u, :], src_ps[u][:], w_sb[:, u:u + 1], rs[:],
                    OP.mult, OP.mult)
                rot = pp_small.tile([128, 512], f32, tag="psm")
                nc.tensor.matmul(rot[:], r0_sb[:], znw[:, u, :], start=True, stop=True)
                nc.vector.tensor_tensor(t1[:, u, :], znw[:, u, :], cos_t, OP.mult)
                tmp = spool.tile([128, 512], f32, tag="tmp")
                nc.vector.tensor_tensor(tmp[:], rot[:], sin_t, OP.mult)
                nc.vector.tensor_tensor(dst[:, dsti + u, :], t1[:, u, :], tmp[:], OP.add)

        for tb in range(NTB):
            t0 = tb * 512
            if tb == 0:
                xth = pre_x
                cos_t = pre_cos
                sin_t = pre_sin
            else:
                xth = []
                for half in range(2):
                    xt = xpool.tile([128, 8, 512], bf16, tag="xt")
                    nc.sync.dma_start(xt[:], xT_v[:, half * 8:(half + 1) * 8, t0:t0 + 512])
                    xth.append(xt)
                cos_t = tabpool.tile([128, 512], f32, tag="cos")
                nc.sync.dma_start(cos_t[:], cosk_d.ap()[:, t0:t0 + 512])
                sin_t = tabpool.tile([128, 512], f32, tag="sin")
                nc.sync.dma_start(sin_t[:], sink_d.ap()[:, t0:t0 + 512])

            # ---- k projection (transposed, N=512) ----
            k0_ps = pp_proj.tile([128, 512], f32, tag="pj")
            k1_ps = pp_proj.tile([128, 512], f32, tag="pj")
            k_ps = [k0_ps, k1_ps]
            for dsub in range(2):
                for ko in range(NKO):
                    nc.tensor.matmul(k_ps[dsub][:],
                                     wk_sb[:, ko, dsub * 128:(dsub + 1) * 128],
                                     xth[ko // 8][:, ko % 8, :],
                                     start=(ko == 0), stop=(ko == NKO - 1))
            kt = kpool.tile([128, 2, 512], bf16, tag="kt")
            norm_rope(k_ps, kw_sb, cos_t[:], sin_t[:], kt, 0)
            kt_tiles[tb] = kt

            # ---- v projection (natural layout) ----
            vt = vpool.tile([128, 4, 256], bf16, tag="vt")
            for vh in range(2):
                v_ps = pp_proj.tile([128, 2, 256], f32, tag="pj")
                for ms in range(2):
                    msub = vh * 2 + ms
                    for ko in range(NKO):
                        nc.tensor.matmul(v_ps[:, ms, :],
                                         xth[ko // 8][:, ko % 8, msub * 128:(msub + 1) * 128],
                                         wv_sb[:, ko, :],
                                         start=(ko == 0), stop=(ko == NKO - 1))
                for ms in range(2):
                    nc.vector.tensor_copy(vt[:, vh * 2 + ms, :], v_ps[:, ms, :])
            vt_tiles[tb] = vt

            if tb < 2:
                continue

            # ---- q projections (2 heads, N=512) ----
            qt_sb = qpool.tile([128, 4, 512], bf16, tag="q")
            for h in range(2):
                q0_ps = pp_proj.tile([128, 512], f32, tag="pj")
                q1_ps = pp_proj.tile([128, 512], f32, tag="pj")
                q_ps = [q0_ps, q1_ps]
                for u in range(2):
                    dsub = 2 * h + u
                    for ko in range(NKO):
                        nc.tensor.matmul(q_ps[u][:],
                                         wq_sb[:, ko, dsub * 128:(dsub + 1) * 128],
                                         xth[ko // 8][:, ko % 8, :],
                                         start=(ko == 0), stop=(ko == NKO - 1))
                norm_rope(q_ps, qw_sb, cos_t[:], sin_t[:], qt_sb, 2 * h)

            # ---- attention for 512-query block a ----
            a = tb - 2
            yt_sb = ypool.tile([128, 4, 512], bf16, tag="y")
            for h in range(2):
                dn_ps = pp_acc.tile([128, 512], f32, tag="pac")
                y0_ps = pp_acc.tile([128, 512], f32, tag="pac")
                y1_ps = pp_acc.tile([128, 512], f32, tag="pac")
                y_ps = [y0_ps, y1_ps]
                for mi, mrel in enumerate([3, 0, 1, 2] + list(range(4, 12))):
                    jt = 4 * a + mrel
                    ct, jh = jt // 4, jt % 4
                    ktc = kt_tiles[ct]
                    vtc = vt_tiles[ct]
                    # active query range: edge tiles are mostly masked
                    if mrel <= 2:
                        ia, ib = 0, 128 * (mrel + 1)
                    elif mrel >= 9:
                        ia, ib = 128 * (mrel - 8), 512
                    else:
                        ia, ib = 0, 512
                    pt = ptpool.tile([128, 512], bf16, tag="p")
                    st = pp_small.tile([128, 512], f32, tag="psm")
                    for u in range(2):
                        nc.tensor.matmul(st[:, ia:ib],
                                         ktc[:, u, jh * 128:(jh + 1) * 128],
                                         qt_sb[:, 2 * h + u, ia:ib],
                                         start=(u == 0), stop=(u == 1))
                    nc.scalar.activation(pt[:, ia:ib], st[:, ia:ib], AF.Exp,
                                         bias=kb_sb[:, jt:jt + 1], scale=SCALE)
                    if mrel < 4:
                        nc.vector.tensor_tensor(
                            pt[:, ia:ib], pt[:, ia:ib],
                            tri_sb[:, mrel * 512 + ia:mrel * 512 + ib], OP.mult)
                    elif mrel >= 8:
                        nc.vector.tensor_tensor(
                            pt[:, ia:ib], pt[:, ia:ib],
                            tri_sb[:, (mrel - 4) * 512 + ia:(mrel - 4) * 512 + ib],
                            OP.mult)
                    first, last = (mi == 0), (mrel == 11)
                    nc.tensor.matmul(dn_ps[:, ia:ib], onesbf_sb[:], pt[:, ia:ib],
                                     start=first, stop=last, skip_group_check=True)
                    for dh in range(2):
                        nc.tensor.matmul(y_ps[dh][:, ia:ib],
                                         vtc[:, jh, dh * 128:(dh + 1) * 128],
                                         pt[:, ia:ib], start=first, stop=last,
                                         skip_group_check=True)
                rc = spool.tile([128, 512], f32, tag="rc")
                nc.vector.reciprocal_approx_fast(rc[:], dn_ps[:])
                for dh in range(2):
                    nc.vector.tensor_tensor(yt_sb[:, 2 * h + dh, :],
                                            y_ps[dh][:], rc[:], OP.mult)

            # ---- partial o-projection for the 512-query block ----
            for msub in range(4):
                for dmh in range(2):
                    o_sb = opool.tile([128, 1024], f32, tag="o")
                    for dq in range(2):
                        c0 = (dmh * 2 + dq) * 512
                        o_ps = pp_small.tile([128, 512], f32, tag="psm")
                        for hd in range(4):
                            nc.tensor.matmul(o_ps[:],
                                             yt_sb[:, hd, msub * 128:(msub + 1) * 128],
                                             wo_sb[:, hd, c0:c0 + 512],
                                             start=(hd == 0), stop=(hd == 3))
                        nc.scalar.copy(o_sb[:, dq * 512:(dq + 1) * 512], o_ps[:])
                    r0_ = a * 512 + msub * 128
                    nc.sync.dma_start(o_d.ap()[r0_:r0_ + 128, dmh * 1024:(dmh + 1) * 1024],
                                      o_sb[:])

    nc.compile()
    _cache["nc"] = nc
    return nc


def _run(inputs, trace=False):
    from concourse.bass_utils import run_bass_kernel_spmd

    nc = _build_program()
    in_maps = _host_prep(**inputs)
    res = run_bass_kernel_spmd(nc, in_maps, core_ids=list(range(8)), trace=trace)
    full = np.zeros((T, DM), np.float32)
    for g in range(NG):
        for s in range(NS):
            full[s * 2048:(s + 1) * 2048] += res.results[g * 2 + s]["o_part"]
    return full.reshape(1, T, DM), res


def kernel(**inputs):
    return _run(inputs, trace=False)[0]



# revision 4
# speedup vs baseline: 1.0431x; 1.0431x over previous
"""Trainium2 Bass kernel for Gemma3 sliding-window attention.

Problem: B=1, T=4096, d_model=2048, 8 query heads / 4 KV heads, head_dim=256,
sliding window 1024, per-head RMSNorm + RoPE (interleaved rotate-half with
cat(freqs,freqs) tables), o_proj.

Sharding (8 cores): 4 KV-head groups x 2 sequence halves. Core (g, s) computes
query heads {2g, 2g+1} and KV head g for query tokens [s*2048, (s+1)*2048),
with a 1024-token KV halo (recomputed locally; s=0's halo is zero-padded and
masked out via the exp bias). Each core emits a partial o-projection
[2048, 2048] in fp16; the host sums the 4 group partials per half.

Dataflow: host pre-transposes x and all weights so every matmul operand loads
in its natural layout (contraction on partitions). Projections and attention
matmuls run in bf16 (fp32 PSUM accumulation); the RMSNorm/RoPE chain runs in
fp32 (ssq via ones-matmul broadcast; rotate-half via a DVE stream_shuffle of
adjacent partitions with a sign-folded sin table — no PE matmul). Attention
is computed in S.T orientation per 512-query block: S.T[j,i] = kT.T @ qT per
128-j tile, P.T = exp(S.T/16 + bias) on ACT (bias -1e5 kills invalid j for
the padded half), constant triangle masks on the 8 window-edge tiles, softmax
denominator via ones-matmul (no max-subtraction: RMSNorm bounds |scores| <=
16), y.T = v.T @ P.T, then y.T is scaled by the reciprocal denominator and
consumed as lhsT by the o-projection. Weight/table DMAs are emitted at their
first-use tile so startup bandwidth goes to wk/wv/x.
"""

import sys

if "/opt/trn_rl_repo" not in sys.path:
    sys.path.insert(0, "/opt/trn_rl_repo")

import numpy as np

try:
    import ml_dtypes
    BF16 = ml_dtypes.bfloat16
except ImportError:
    BF16 = None

T, DM, NH, NKV, HD, WIN = 4096, 2048, 8, 4, 256, 1024
EPS, BASE = 1e-6, 10000.0
NG, NS = 4, 2
TL, NQ = 3072, 2048
NKO = 16          # 2048 / 128 contraction subtiles
SCALE = 1.0 / 16.0
NEG = -1.0e5
SWAP_MASK = [m ^ 1 for m in range(32)]   # pairwise partition swap

_cache = {}


def _host_prep(x, pos, Wq, Wk, Wv, Wo, q_norm_w, k_norm_w):
    x = np.asarray(x, np.float32).reshape(T, DM)
    xT = np.ascontiguousarray(x.T)
    pos_f = np.asarray(pos).astype(np.float64)
    m = np.arange(128)
    invf = BASE ** (-m / 128.0)

    Wq = np.asarray(Wq, np.float32)
    Wk = np.asarray(Wk, np.float32)
    Wv = np.asarray(Wv, np.float32)
    Wo = np.asarray(Wo, np.float32)
    qnw = np.asarray(q_norm_w, np.float32)
    knw = np.asarray(k_norm_w, np.float32)

    ones = np.ones((128, 128), np.float32)
    qw2 = np.ascontiguousarray(np.stack([qnw[:128], qnw[128:]], axis=1))
    kw2 = np.ascontiguousarray(np.stack([knw[:128], knw[128:]], axis=1))
    # sign fold for rotate-half via partition swap: out[2a]   -= z[2a+1]*sin
    #                                              out[2a+1] += z[2a]  *sin
    sgn = np.where(m % 2 == 0, -1.0, 1.0)[:, None]

    # masks for 512-wide attention blocks: m=0..3 far edge, m=4..7 diagonal
    jp = np.arange(128)[:, None]
    ip = np.arange(512)[None, :]
    tris = []
    for mm_ in range(4):
        tris.append(jp >= ip + 1 - 128 * mm_)         # far masks F_m
    for mm_ in range(4):
        tris.append(jp <= ip - 128 * mm_)             # diag masks D_{m+8}
    tri = np.concatenate(tris, axis=1).astype(BF16)   # [128, 8*512]

    in_maps = []
    for g in range(NG):
        for s in range(NS):
            lo = s * 2048 - 1024
            xT_c = np.zeros((DM, TL), np.float32)
            src_lo = max(lo, 0)
            xT_c[:, src_lo - lo:] = xT[:, src_lo:(s + 1) * 2048]
            pidx = np.clip(np.arange(lo, lo + TL), 0, T - 1)
            p = pos_f[pidx]
            p[np.arange(lo, lo + TL) < 0] = 0.0
            ang = p[None, :] * invf[:, None]
            cosk = np.ascontiguousarray(np.cos(ang)).astype(BF16)
            sink = np.ascontiguousarray(np.sin(ang) * sgn).astype(BF16)

            kbias = np.zeros((128, 24), np.float32)
            if s == 0:
                kbias[:, :8] = NEG

            in_maps.append({
                "xT": xT_c.astype(BF16),
                "cosk": cosk,
                "sink": sink,
                "wqT": np.ascontiguousarray(Wq[2 * g * HD:(2 * g + 2) * HD, :].T).astype(BF16),
                "wkT": np.ascontiguousarray(Wk[g * HD:(g + 1) * HD, :].T).astype(BF16),
                "wvT": np.ascontiguousarray(Wv[g * HD:(g + 1) * HD, :].T).astype(BF16),
                "woT": np.ascontiguousarray(Wo[:, 2 * g * HD:(2 * g + 2) * HD].T).astype(BF16),
                "ones_bf": ones.astype(BF16),
                "qw": qw2,
                "kw": kw2,
                "kbias": kbias,
                "tri": tri,
            })
    return in_maps


def _build_program():
    if "nc" in _cache:
        return _cache["nc"]

    import concourse.bass as bass
    import concourse.mybir as mybir
    import concourse.tile as tile
    from concourse import bacc
    from contextlib import ExitStack

    f32 = mybir.dt.float32
    f16 = mybir.dt.float16
    bf16 = mybir.dt.bfloat16
    AF = mybir.ActivationFunctionType
    OP = mybir.AluOpType

    nc = bacc.Bacc("TRN2", target_bir_lowering=False, debug=False,
                   enable_asserts=False, num_devices=8)

    xT_d = nc.dram_tensor("xT", [DM, TL], bf16, kind="ExternalInput")
    cosk_d = nc.dram_tensor("cosk", [128, TL], bf16, kind="ExternalInput")
    sink_d = nc.dram_tensor("sink", [128, TL], bf16, kind="ExternalInput")
    wq_d = nc.dram_tensor("wqT", [DM, 512], bf16, kind="ExternalInput")
    wk_d = nc.dram_tensor("wkT", [DM, 256], bf16, kind="ExternalInput")
    wv_d = nc.dram_tensor("wvT", [DM, 256], bf16, kind="ExternalInput")
    wo_d = nc.dram_tensor("woT", [512, DM], bf16, kind="ExternalInput")
    onesbf_d = nc.dram_tensor("ones_bf", [128, 128], bf16, kind="ExternalInput")
    qw_d = nc.dram_tensor("qw", [128, 2], f32, kind="ExternalInput")
    kw_d = nc.dram_tensor("kw", [128, 2], f32, kind="ExternalInput")
    kb_d = nc.dram_tensor("kbias", [128, 24], f32, kind="ExternalInput")
    tri_d = nc.dram_tensor("tri", [128, 8 * 512], bf16, kind="ExternalInput")
    o_d = nc.dram_tensor("o_part", [NQ, DM], f16, kind="ExternalOutput")

    with tile.TileContext(nc) as tc, ExitStack() as ctx:
        cpool = ctx.enter_context(tc.tile_pool(name="consts", bufs=1))
        xpool = ctx.enter_context(tc.tile_pool(name="xt", bufs=4))
        tabpool = ctx.enter_context(tc.tile_pool(name="tab", bufs=2))
        kpool = ctx.enter_context(tc.tile_pool(name="kring", bufs=5))
        vpool = ctx.enter_context(tc.tile_pool(name="vring", bufs=5))
        scpool = ctx.enter_context(tc.tile_pool(name="scratch", bufs=3))
        spool = ctx.enter_context(tc.tile_pool(name="small", bufs=2))
        qpool = ctx.enter_context(tc.tile_pool(name="qt", bufs=3))
        ptpool = ctx.enter_context(tc.tile_pool(name="pt", bufs=4))
        ypool = ctx.enter_context(tc.tile_pool(name="yt", bufs=3))
        opool = ctx.enter_context(tc.tile_pool(name="osb", bufs=3))
        pp_proj = ctx.enter_context(tc.tile_pool(name="pproj", bufs=3, space="PSUM"))
        pp_small = ctx.enter_context(tc.tile_pool(name="psmall", bufs=2, space="PSUM"))
        pp_acc = ctx.enter_context(tc.tile_pool(name="pacc", bufs=3, space="PSUM"))

        # ---- early constants: only what tb=0/1 k/v projections + norms need.
        # wq/tri/wo are deferred to their first-use tiles so the startup DMA
        # bandwidth goes to wk/wv/x0.
        wk_sb = cpool.tile([128, NKO, 256], bf16, tag="wk")
        nc.sync.dma_start(wk_sb[:], wk_d.ap().rearrange("(ko p) c -> p ko c", p=128))
        wv_sb = cpool.tile([128, NKO, 256], bf16, tag="wv")
        nc.sync.dma_start(wv_sb[:], wv_d.ap().rearrange("(ko p) c -> p ko c", p=128))
        ones_sb = cpool.tile([128, 128], bf16, tag="ones")
        nc.sync.dma_start(ones_sb[:], onesbf_d.ap())
        onesbf_sb = ones_sb
        qw_sb = cpool.tile([128, 2], f32, tag="qwt")
        nc.sync.dma_start(qw_sb[:], qw_d.ap())
        kw_sb = cpool.tile([128, 2], f32, tag="kwt")
        nc.sync.dma_start(kw_sb[:], kw_d.ap())
        kb_sb = cpool.tile([128, 24], f32, tag="kb")
        nc.sync.dma_start(kb_sb[:], kb_d.ap())
        eps_sb = cpool.tile([128, 1], f32, tag="eps")
        nc.vector.memset(eps_sb[:], EPS)
        zero_sb = cpool.tile([128, 1], f32, tag="zero")
        nc.vector.memset(zero_sb[:], 0.0)
        xT_v = xT_d.ap().rearrange("(ko p) t -> p ko t", p=128)  # [128, 16, TL]
        pre_x = []
        for half in range(2):
            xt0 = xpool.tile([128, 8, 512], bf16, tag="xt")
            nc.sync.dma_start(xt0[:], xT_v[:, half * 8:(half + 1) * 8, 0:512])
            pre_x.append(xt0)
        pre_cos = tabpool.tile([128, 512], bf16, tag="cos")
        nc.sync.dma_start(pre_cos[:], cosk_d.ap()[:, 0:512])
        pre_sin = tabpool.tile([128, 512], bf16, tag="sin")
        nc.sync.dma_start(pre_sin[:], sink_d.ap()[:, 0:512])
        # deferred weights (emitted inside the tile loop):
        wq_sb = cpool.tile([128, NKO, 512], bf16, tag="wq")
        tri_sb = cpool.tile([128, 8 * 512], bf16, tag="tri")
        wo_sb = cpool.tile([128, 4, DM], bf16, tag="wo")

        NTB = 6                     # 512-token projection tiles
        kt_tiles = [None] * NTB
        vt_tiles = [None] * NTB

        def norm_rope(src_ps, w_sb, cos_t, sin_t, dst, dsti):
            """src_ps: two PSUM [128, 512] tiles (one head's 2 d-subtiles),
            transposed projection over 512 tokens. Writes RMSNorm+RoPE (bf16)
            into dst[:, dsti+u, :]. Rotate-half = partition swap (DVE
            stream_shuffle) with the sign folded into the sin table."""
            z2 = scpool.tile([128, 2, 512], bf16, tag="z2")
            for u in range(2):
                nc.scalar.activation(z2[:, u, :], src_ps[u][:], AF.Square,
                                     bias=zero_sb[:])
            ssq = pp_small.tile([128, 512], f32, tag="psm")
            for u in range(2):
                nc.tensor.matmul(ssq[:], ones_sb[:], z2[:, u, :],
                                 start=(u == 0), stop=(u == 1))
            sq = spool.tile([128, 512], f32, tag="sq")
            nc.scalar.activation(sq[:], ssq[:], AF.Sqrt, bias=eps_sb[:], scale=1.0 / HD)
            rs = spool.tile([128, 512], f32, tag="rs")
            nc.vector.reciprocal_approx_fast(rs[:], sq[:])
            znw = scpool.tile([128, 2, 512], bf16, tag="znw")
            t1 = scpool.tile([128, 2, 512], f32, tag="t1")
            for u in range(2):
                nc.vector.scalar_tensor_tensor(
                    znw[:, u, :], src_ps[u][:], w_sb[:, u:u + 1], rs[:],
                    OP.mult, OP.mult)
                sh = spool.tile([128, 512], bf16, tag="sh")
                nc.vector.stream_shuffle(sh[:], znw[:, u, :], SWAP_MASK)
                nc.vector.tensor_tensor(t1[:, u, :], znw[:, u, :], cos_t, OP.mult)
                tmp = spool.tile([128, 512], f32, tag="tmp")
                nc.vector.tensor_tensor(tmp[:], sh[:], sin_t, OP.mult)
                nc.vector.tensor_tensor(dst[:, dsti + u, :], t1[:, u, :], tmp[:], OP.add)

        for tb in range(NTB):
            t0 = tb * 512
            if tb == 0:
                xth = pre_x
                cos_t = pre_cos
                sin_t = pre_sin
            else:
                xth = []
                for half in range(2):
                    xt = xpool.tile([128, 8, 512], bf16, tag="xt")
                    nc.sync.dma_start(xt[:], xT_v[:, half * 8:(half + 1) * 8, t0:t0 + 512])
                    xth.append(xt)
                cos_t = tabpool.tile([128, 512], bf16, tag="cos")
                nc.sync.dma_start(cos_t[:], cosk_d.ap()[:, t0:t0 + 512])
                sin_t = tabpool.tile([128, 512], bf16, tag="sin")
                nc.sync.dma_start(sin_t[:], sink_d.ap()[:, t0:t0 + 512])

            if tb == 1:
                # first-use deferral: wq+tri needed from tb=2
                nc.sync.dma_start(wq_sb[:], wq_d.ap().rearrange("(ko p) c -> p ko c", p=128))
                nc.sync.dma_start(tri_sb[:], tri_d.ap())
            elif tb == 2:
                # wo needed at the first o-projection (end of tb=2)
                nc.sync.dma_start(wo_sb[:], wo_d.ap().rearrange("(hd p) c -> p hd c", p=128))

            # ---- k projection (transposed, N=512) ----
            k0_ps = pp_proj.tile([128, 512], f32, tag="pj")
            k1_ps = pp_proj.tile([128, 512], f32, tag="pj")
            k_ps = [k0_ps, k1_ps]
            for dsub in range(2):
                for ko in range(NKO):
                    nc.tensor.matmul(k_ps[dsub][:],
                                     wk_sb[:, ko, dsub * 128:(dsub + 1) * 128],
                                     xth[ko // 8][:, ko % 8, :],
                                     start=(ko == 0), stop=(ko == NKO - 1))
            kt = kpool.tile([128, 2, 512], bf16, tag="kt")
            norm_rope(k_ps, kw_sb, cos_t[:], sin_t[:], kt, 0)
            kt_tiles[tb] = kt

            # ---- v projection (natural layout) ----
            vt = vpool.tile([128, 4, 256], bf16, tag="vt")
            for vh in range(2):
                v_ps = pp_proj.tile([128, 2, 256], f32, tag="pj")
                for ms in range(2):
                    msub = vh * 2 + ms
                    for ko in range(NKO):
                        nc.tensor.matmul(v_ps[:, ms, :],
                                         xth[ko // 8][:, ko % 8, msub * 128:(msub + 1) * 128],
                                         wv_sb[:, ko, :],
                                         start=(ko == 0), stop=(ko == NKO - 1))
                for ms in range(2):
                    nc.vector.tensor_copy(vt[:, vh * 2 + ms, :], v_ps[:, ms, :])
            vt_tiles[tb] = vt

            if tb < 2:
                continue

            # ---- q projections (2 heads, N=512) ----
            qt_sb = qpool.tile([128, 4, 512], bf16, tag="q")
            for h in range(2):
                q0_ps = pp_proj.tile([128, 512], f32, tag="pj")
                q1_ps = pp_proj.tile([128, 512], f32, tag="pj")
                q_ps = [q0_ps, q1_ps]
                for u in range(2):
                    dsub = 2 * h + u
                    for ko in range(NKO):
                        nc.tensor.matmul(q_ps[u][:],
                                         wq_sb[:, ko, dsub * 128:(dsub + 1) * 128],
                                         xth[ko // 8][:, ko % 8, :],
                                         start=(ko == 0), stop=(ko == NKO - 1))
                norm_rope(q_ps, qw_sb, cos_t[:], sin_t[:], qt_sb, 2 * h)

            # ---- attention for 512-query block a ----
            a = tb - 2
            yt_sb = ypool.tile([128, 4, 512], bf16, tag="y")
            for h in range(2):
                dn_ps = pp_acc.tile([128, 512], f32, tag="pac")
                y0_ps = pp_acc.tile([128, 512], f32, tag="pac")
                y1_ps = pp_acc.tile([128, 512], f32, tag="pac")
                y_ps = [y0_ps, y1_ps]
                for mi, mrel in enumerate([3, 0, 1, 2] + list(range(4, 12))):
                    jt = 4 * a + mrel
                    ct, jh = jt // 4, jt % 4
                    ktc = kt_tiles[ct]
                    vtc = vt_tiles[ct]
                    # active query range: edge tiles are mostly masked
                    if mrel <= 2:
                        ia, ib = 0, 128 * (mrel + 1)
                    elif mrel >= 9:
                        ia, ib = 128 * (mrel - 8), 512
                    else:
                        ia, ib = 0, 512
                    pt = ptpool.tile([128, 512], bf16, tag="p")
                    st = pp_small.tile([128, 512], f32, tag="psm")
                    for u in range(2):
                        nc.tensor.matmul(st[:, ia:ib],
                                         ktc[:, u, jh * 128:(jh + 1) * 128],
                                         qt_sb[:, 2 * h + u, ia:ib],
                                         start=(u == 0), stop=(u == 1))
                    nc.scalar.activation(pt[:, ia:ib], st[:, ia:ib], AF.Exp,
                                         bias=kb_sb[:, jt:jt + 1], scale=SCALE)
                    if mrel < 4:
                        nc.vector.tensor_tensor(
                            pt[:, ia:ib], pt[:, ia:ib],
                            tri_sb[:, mrel * 512 + ia:mrel * 512 + ib], OP.mult)
                    elif mrel >= 8:
                        nc.vector.tensor_tensor(
                            pt[:, ia:ib], pt[:, ia:ib],
                            tri_sb[:, (mrel - 4) * 512 + ia:(mrel - 4) * 512 + ib],
                            OP.mult)
                    first, last = (mi == 0), (mrel == 11)
                    nc.tensor.matmul(dn_ps[:, ia:ib], onesbf_sb[:], pt[:, ia:ib],
                                     start=first, stop=last, skip_group_check=True)
                    for dh in range(2):
                        nc.tensor.matmul(y_ps[dh][:, ia:ib],
                                         vtc[:, jh, dh * 128:(dh + 1) * 128],
                                         pt[:, ia:ib], start=first, stop=last,
                                         skip_group_check=True)
                rc = spool.tile([128, 512], f32, tag="rc")
                nc.vector.reciprocal_approx_fast(rc[:], dn_ps[:])
                for dh in range(2):
                    nc.vector.tensor_tensor(yt_sb[:, 2 * h + dh, :],
                                            y_ps[dh][:], rc[:], OP.mult)

            # ---- partial o-projection for the 512-query block ----
            for msub in range(4):
                for dmh in range(2):
                    o_sb = opool.tile([128, 1024], f16, tag="o")
                    for dq in range(2):
                        c0 = (dmh * 2 + dq) * 512
                        o_ps = pp_small.tile([128, 512], f32, tag="psm")
                        for hd in range(4):
                            nc.tensor.matmul(o_ps[:],
                                             yt_sb[:, hd, msub * 128:(msub + 1) * 128],
                                             wo_sb[:, hd, c0:c0 + 512],
                                             start=(hd == 0), stop=(hd == 3))
                        nc.scalar.copy(o_sb[:, dq * 512:(dq + 1) * 512], o_ps[:])
                    r0_ = a * 512 + msub * 128
                    nc.sync.dma_start(o_d.ap()[r0_:r0_ + 128, dmh * 1024:(dmh + 1) * 1024],
                                      o_sb[:])

    nc.compile()
    _cache["nc"] = nc
    return nc


def _run(inputs, trace=False):
    from concourse.bass_utils import run_bass_kernel_spmd

    nc = _build_program()
    in_maps = _host_prep(**inputs)
    res = run_bass_kernel_spmd(nc, in_maps, core_ids=list(range(8)), trace=trace)
    full = np.zeros((T, DM), np.float32)
    for g in range(NG):
        for s in range(NS):
            full[s * 2048:(s + 1) * 2048] += res.results[g * 2 + s]["o_part"].astype(np.float32)
    return full.reshape(1, T, DM), res


def kernel(**inputs):
    return _run(inputs, trace=False)[0]


# revision 13
# speedup vs baseline: 1.1476x; 1.1002x over previous
"""Trainium2 Bass kernel for Gemma3 sliding-window attention.

Problem: B=1, T=4096, d_model=2048, 8 query heads / 4 KV heads, head_dim=256,
sliding window 1024, per-head RMSNorm + RoPE (interleaved rotate-half with
cat(freqs,freqs) tables), o_proj.

Sharding (8 cores): 4 KV-head groups x 2 sequence halves. Core (g, s) computes
query heads {2g, 2g+1} and KV head g for query tokens [s*2048, (s+1)*2048).
The 1024-token KV halo is NOT recomputed: the two cores of a group exchange
their post-norm/RoPE k/v tiles for tokens [1024, 2048) (local tiles 4,5) via
a pair AllGather ([[0,1],[2,3],[4,5],[6,7]]); both cores read gather slot 0
(= core (g,0)'s tiles). For (g,1) that is exactly its halo; for (g,0) the
received data is its own tokens re-read, which the exp bias (-1e5 on local
key tiles 0..7) fully masks — so the SPMD program stays divergence-free.
Local k/v tiles are produced in order [4,5,2,3] so the exchange is issued
early; attention blocks run [2,3,0,1] so the first two need only local
tiles, hiding the collective latency entirely.

Dataflow: host pre-transposes x and all weights so every matmul operand loads
in its natural layout. Projections and attention matmuls run in bf16 (fp32
PSUM); RMSNorm/RoPE run on ACT/DVE (ssq via ones-matmul broadcast;
rotate-half via a DVE stream_shuffle of adjacent partitions with a
sign-folded sin table). Attention is computed in S.T orientation per
512-query block (see norm_rope/attention code). Each block's partial
o-projection is emitted one block late, filling the q-norm latency bubble
with o-proj matmuls. Partial outputs are fp16; the host sums the 4 group
partials per half in fp32.
"""

import sys

if "/opt/trn_rl_repo" not in sys.path:
    sys.path.insert(0, "/opt/trn_rl_repo")

import numpy as np

try:
    import ml_dtypes
    BF16 = ml_dtypes.bfloat16
except ImportError:
    BF16 = None

T, DM, NH, NKV, HD, WIN = 4096, 2048, 8, 4, 256, 1024
EPS, BASE = 1e-6, 10000.0
NG, NS = 4, 2
TL, NQ = 3072, 2048
NKO = 16          # 2048 / 128 contraction subtiles
SCALE = 1.0 / 16.0
NEG = -1.0e5
SWAP_MASK = [m ^ 1 for m in range(32)]   # pairwise partition swap
HALO_CC = True    # exchange halo k/v via pair AllGather (False: recompute)

_cache = {}


def _host_prep(x, pos, Wq, Wk, Wv, Wo, q_norm_w, k_norm_w):
    x = np.asarray(x, np.float32).reshape(T, DM)
    xT = np.ascontiguousarray(x.T)
    pos_f = np.asarray(pos).astype(np.float64)
    m = np.arange(128)
    invf = BASE ** (-m / 128.0)

    Wq = np.asarray(Wq, np.float32)
    Wk = np.asarray(Wk, np.float32)
    Wv = np.asarray(Wv, np.float32)
    Wo = np.asarray(Wo, np.float32)
    qnw = np.asarray(q_norm_w, np.float32)
    knw = np.asarray(k_norm_w, np.float32)

    ones = np.ones((128, 128), np.float32)
    qw2 = np.ascontiguousarray(np.stack([qnw[:128], qnw[128:]], axis=1))
    kw2 = np.ascontiguousarray(np.stack([knw[:128], knw[128:]], axis=1))
    sgn = np.where(m % 2 == 0, -1.0, 1.0)[:, None]

    jp = np.arange(128)[:, None]
    ip = np.arange(512)[None, :]
    tris = []
    for mm_ in range(4):
        tris.append(jp >= ip + 1 - 128 * mm_)         # far masks F_m
    for mm_ in range(4):
        tris.append(jp <= ip - 128 * mm_)             # diag masks D_{m+8}
    tri = np.concatenate(tris, axis=1).astype(BF16)   # [128, 8*512]

    in_maps = []
    for g in range(NG):
        for s in range(NS):
            lo = s * 2048 - 1024
            xT_c = np.zeros((DM, TL), np.float32)
            src_lo = max(lo, 0)
            xT_c[:, src_lo - lo:] = xT[:, src_lo:(s + 1) * 2048]
            pidx = np.clip(np.arange(lo, lo + TL), 0, T - 1)
            p = pos_f[pidx]
            p[np.arange(lo, lo + TL) < 0] = 0.0
            ang = p[None, :] * invf[:, None]
            cosk = np.ascontiguousarray(np.cos(ang)).astype(BF16)
            sink = np.ascontiguousarray(np.sin(ang) * sgn).astype(BF16)

            kbias = np.zeros((128, 24), np.float32)
            if s == 0:
                kbias[:, :8] = NEG

            in_maps.append({
                "xT": xT_c.astype(BF16),
                "cosk": cosk,
                "sink": sink,
                "wqT": np.ascontiguousarray(Wq[2 * g * HD:(2 * g + 2) * HD, :].T).astype(BF16),
                "wkT": np.ascontiguousarray(Wk[g * HD:(g + 1) * HD, :].T).astype(BF16),
                "wvT": np.ascontiguousarray(Wv[g * HD:(g + 1) * HD, :].T).astype(BF16),
                "woT": np.ascontiguousarray(Wo[:, 2 * g * HD:(2 * g + 2) * HD].T).astype(BF16),
                "ones_bf": ones.astype(BF16),
                "qw": qw2,
                "kw": kw2,
                "kbias": kbias,
                "tri": tri,
            })
    return in_maps


def _build_program():
    if "nc" in _cache:
        return _cache["nc"]

    import concourse.bass as bass
    import concourse.mybir as mybir
    import concourse.tile as tile
    from concourse import bacc
    from contextlib import ExitStack

    f32 = mybir.dt.float32
    f16 = mybir.dt.float16
    bf16 = mybir.dt.bfloat16
    AF = mybir.ActivationFunctionType
    OP = mybir.AluOpType

    nc = bacc.Bacc("TRN2", target_bir_lowering=False, debug=False,
                   enable_asserts=False, num_devices=8)

    xT_d = nc.dram_tensor("xT", [DM, TL], bf16, kind="ExternalInput")
    cosk_d = nc.dram_tensor("cosk", [128, TL], bf16, kind="ExternalInput")
    sink_d = nc.dram_tensor("sink", [128, TL], bf16, kind="ExternalInput")
    wq_d = nc.dram_tensor("wqT", [DM, 512], bf16, kind="ExternalInput")
    wk_d = nc.dram_tensor("wkT", [DM, 256], bf16, kind="ExternalInput")
    wv_d = nc.dram_tensor("wvT", [DM, 256], bf16, kind="ExternalInput")
    wo_d = nc.dram_tensor("woT", [512, DM], bf16, kind="ExternalInput")
    onesbf_d = nc.dram_tensor("ones_bf", [128, 128], bf16, kind="ExternalInput")
    qw_d = nc.dram_tensor("qw", [128, 2], f32, kind="ExternalInput")
    kw_d = nc.dram_tensor("kw", [128, 2], f32, kind="ExternalInput")
    kb_d = nc.dram_tensor("kbias", [128, 24], f32, kind="ExternalInput")
    tri_d = nc.dram_tensor("tri", [128, 8 * 512], bf16, kind="ExternalInput")
    o_d = nc.dram_tensor("o_part", [NQ, DM], f16, kind="ExternalOutput")

    if HALO_CC:
        # halo exchange bounce buffers: 4 tiles (kt4, kt5, vt4, vt5)
        hx_in = nc.dram_tensor("hx_in", [4, 128, 1024], bf16)
        hx_out = nc.dram_tensor("hx_out", [2, 4, 128, 1024], bf16)

    with tile.TileContext(nc) as tc, ExitStack() as ctx:
        cpool = ctx.enter_context(tc.tile_pool(name="consts", bufs=1))
        xpool = ctx.enter_context(tc.tile_pool(name="xt", bufs=4))
        tabpool = ctx.enter_context(tc.tile_pool(name="tab", bufs=8 if HALO_CC else 12))
        kpool = ctx.enter_context(tc.tile_pool(name="kring", bufs=6))
        vpool = ctx.enter_context(tc.tile_pool(name="vring", bufs=6))
        scpool = ctx.enter_context(tc.tile_pool(name="scratch", bufs=3))
        spool = ctx.enter_context(tc.tile_pool(name="small", bufs=2))
        qpool = ctx.enter_context(tc.tile_pool(name="qt", bufs=3))
        ptpool = ctx.enter_context(tc.tile_pool(name="pt", bufs=4))
        ypool = ctx.enter_context(tc.tile_pool(name="yt", bufs=3))
        opool = ctx.enter_context(tc.tile_pool(name="osb", bufs=3))
        pp_proj = ctx.enter_context(tc.tile_pool(name="pproj", bufs=3, space="PSUM"))
        pp_small = ctx.enter_context(tc.tile_pool(name="psmall", bufs=2, space="PSUM"))
        pp_acc = ctx.enter_context(tc.tile_pool(name="pacc", bufs=3, space="PSUM"))

        xT_v = xT_d.ap().rearrange("(ko p) t -> p ko t", p=128)  # [128, 16, TL]
        x_tiles = {}
        tabs = {}

        def load_x_halves(t):
            t0 = t * 512
            halves = []
            for half in range(2):
                xt = xpool.tile([128, 8, 512], bf16, tag="xt")
                nc.sync.dma_start(xt[:], xT_v[:, half * 8:(half + 1) * 8, t0:t0 + 512])
                halves.append(xt)
            return halves

        def load_x(t):
            t0 = t * 512
            x_tiles[t] = load_x_halves(t)
            cos_t = tabpool.tile([128, 512], bf16, tag="cos")
            nc.sync.dma_start(cos_t[:], cosk_d.ap()[:, t0:t0 + 512])
            sin_t = tabpool.tile([128, 512], bf16, tag="sin")
            nc.sync.dma_start(sin_t[:], sink_d.ap()[:, t0:t0 + 512])
            tabs[t] = (cos_t, sin_t)

        # ---- startup: interleave x-tile-4 with wk/wv so the first k-proj
        # starts as soon as possible; everything else follows.
        wk_sb = cpool.tile([128, NKO, 256], bf16, tag="wk")
        wv_sb = cpool.tile([128, NKO, 256], bf16, tag="wv")
        wk_v = wk_d.ap().rearrange("(ko p) c -> p ko c", p=128)
        wv_v = wv_d.ap().rearrange("(ko p) c -> p ko c", p=128)
        t0_first = 4 * 512
        xt4 = []
        xt4_0 = xpool.tile([128, 8, 512], bf16, tag="xt")
        nc.sync.dma_start(xt4_0[:], xT_v[:, 0:8, t0_first:t0_first + 512])
        xt4.append(xt4_0)
        nc.sync.dma_start(wk_sb[:, 0:8, :], wk_v[:, 0:8, :])
        xt4_1 = xpool.tile([128, 8, 512], bf16, tag="xt")
        nc.sync.dma_start(xt4_1[:], xT_v[:, 8:16, t0_first:t0_first + 512])
        xt4.append(xt4_1)
        nc.sync.dma_start(wk_sb[:, 8:16, :], wk_v[:, 8:16, :])
        x_tiles[4] = xt4
        nc.sync.dma_start(wv_sb[:, 0:8, :], wv_v[:, 0:8, :])
        nc.sync.dma_start(wv_sb[:, 8:16, :], wv_v[:, 8:16, :])
        cos4 = tabpool.tile([128, 512], bf16, tag="cos")
        nc.sync.dma_start(cos4[:], cosk_d.ap()[:, t0_first:t0_first + 512])
        sin4 = tabpool.tile([128, 512], bf16, tag="sin")
        nc.sync.dma_start(sin4[:], sink_d.ap()[:, t0_first:t0_first + 512])
        tabs[4] = (cos4, sin4)
        ones_sb = cpool.tile([128, 128], bf16, tag="ones")
        nc.sync.dma_start(ones_sb[:], onesbf_d.ap())
        onesbf_sb = ones_sb
        qw_sb = cpool.tile([128, 2], f32, tag="qwt")
        nc.sync.dma_start(qw_sb[:], qw_d.ap())
        kw_sb = cpool.tile([128, 2], f32, tag="kwt")
        nc.sync.dma_start(kw_sb[:], kw_d.ap())
        kb_sb = cpool.tile([128, 24], f32, tag="kb")
        nc.sync.dma_start(kb_sb[:], kb_d.ap())
        eps_sb = cpool.tile([128, 1], f32, tag="eps")
        nc.vector.memset(eps_sb[:], EPS)
        zero_sb = cpool.tile([128, 1], f32, tag="zero")
        nc.vector.memset(zero_sb[:], 0.0)
        load_x(5)
        # deferred weights (DMAs emitted at first-use phases below):
        wq_sb = cpool.tile([128, NKO, 512], bf16, tag="wq")
        tri_sb = cpool.tile([128, 8 * 512], bf16, tag="tri")
        wo_sb = cpool.tile([128, 4, DM], bf16, tag="wo")

        kt_tiles = [None] * 6
        vt_tiles = [None] * 6

        def norm_rope(src_ps, w_sb, cos_t, sin_t, dst, dsti):
            z2 = scpool.tile([128, 2, 512], bf16, tag="z2")
            for u in range(2):
                nc.scalar.activation(z2[:, u, :], src_ps[u][:], AF.Square,
                                     bias=zero_sb[:])
            ssq = pp_small.tile([128, 512], f32, tag="psm")
            for u in range(2):
                nc.tensor.matmul(ssq[:], ones_sb[:], z2[:, u, :],
                                 start=(u == 0), stop=(u == 1))
            rs = spool.tile([128, 512], f32, tag="rs")
            nc.scalar.activation(rs[:], ssq[:], AF.Rsqrt, bias=eps_sb[:], scale=1.0 / HD)
            znw = scpool.tile([128, 2, 512], bf16, tag="znw")
            t1 = scpool.tile([128, 2, 512], f32, tag="t1")
            for u in range(2):
                nc.vector.scalar_tensor_tensor(
                    znw[:, u, :], src_ps[u][:], w_sb[:, u:u + 1], rs[:],
                    OP.mult, OP.mult)
                sh = spool.tile([128, 512], bf16, tag="sh")
                nc.vector.stream_shuffle(sh[:], znw[:, u, :], SWAP_MASK)
                nc.vector.tensor_tensor(t1[:, u, :], znw[:, u, :], cos_t, OP.mult)
                tmp = spool.tile([128, 512], f32, tag="tmp")
                nc.vector.tensor_tensor(tmp[:], sh[:], sin_t, OP.mult)
                nc.vector.tensor_tensor(dst[:, dsti + u, :], t1[:, u, :], tmp[:], OP.add)

        def kv_proj(t):
            xth = x_tiles[t]
            cos_t, sin_t = tabs[t]
            k0_ps = pp_proj.tile([128, 512], f32, tag="pj")
            k1_ps = pp_proj.tile([128, 512], f32, tag="pj")
            k_ps = [k0_ps, k1_ps]
            for dsub in range(2):
                for ko in range(NKO):
                    nc.tensor.matmul(k_ps[dsub][:],
                                     wk_sb[:, ko, dsub * 128:(dsub + 1) * 128],
                                     xth[ko // 8][:, ko % 8, :],
                                     start=(ko == 0), stop=(ko == NKO - 1))
            kt = kpool.tile([128, 2, 512], bf16, tag="kt")
            norm_rope(k_ps, kw_sb, cos_t[:], sin_t[:], kt, 0)
            kt_tiles[t] = kt

            vt = vpool.tile([128, 4, 256], bf16, tag="vt")
            for vh in range(2):
                v_ps = pp_proj.tile([128, 2, 256], f32, tag="pj")
                for ms in range(2):
                    msub = vh * 2 + ms
                    for ko in range(NKO):
                        nc.tensor.matmul(v_ps[:, ms, :],
                                         xth[ko // 8][:, ko % 8, msub * 128:(msub + 1) * 128],
                                         wv_sb[:, ko, :],
                                         start=(ko == 0), stop=(ko == NKO - 1))
                for ms in range(2):
                    nc.vector.tensor_copy(vt[:, vh * 2 + ms, :], v_ps[:, ms, :])
            vt_tiles[t] = vt

        def q_proj(a, xth):
            cos_t, sin_t = tabs[a + 2]
            qt_sb = qpool.tile([128, 4, 512], bf16, tag="q")
            for h in range(2):
                q0_ps = pp_proj.tile([128, 512], f32, tag="pj")
                q1_ps = pp_proj.tile([128, 512], f32, tag="pj")
                q_ps = [q0_ps, q1_ps]
                for u in range(2):
                    dsub = 2 * h + u
                    for ko in range(NKO):
                        nc.tensor.matmul(q_ps[u][:],
                                         wq_sb[:, ko, dsub * 128:(dsub + 1) * 128],
                                         xth[ko // 8][:, ko % 8, :],
                                         start=(ko == 0), stop=(ko == NKO - 1))
                norm_rope(q_ps, qw_sb, cos_t[:], sin_t[:], qt_sb, 2 * h)
            return qt_sb

        def attention(a, qt_sb):
            yt_sb = ypool.tile([128, 4, 512], bf16, tag="y")
            for h in range(2):
                dn_ps = pp_acc.tile([128, 512], f32, tag="pac")
                y0_ps = pp_acc.tile([128, 512], f32, tag="pac")
                y1_ps = pp_acc.tile([128, 512], f32, tag="pac")
                y_ps = [y0_ps, y1_ps]
                for mi, mrel in enumerate([3, 0, 1, 2] + list(range(4, 12))):
                    jt = 4 * a + mrel
                    ct, jh = jt // 4, jt % 4
                    ktc = kt_tiles[ct]
                    vtc = vt_tiles[ct]
                    if mrel <= 2:
                        ia, ib = 0, 128 * (mrel + 1)
                    elif mrel >= 9:
                        ia, ib = 128 * (mrel - 8), 512
                    else:
                        ia, ib = 0, 512
                    pt = ptpool.tile([128, 512], bf16, tag="p")
                    st = pp_small.tile([128, 512], f32, tag="psm")
                    for u in range(2):
                        nc.tensor.matmul(st[:, ia:ib],
                                         ktc[:, u, jh * 128:(jh + 1) * 128],
                                         qt_sb[:, 2 * h + u, ia:ib],
                                         start=(u == 0), stop=(u == 1))
                    nc.scalar.activation(pt[:, ia:ib], st[:, ia:ib], AF.Exp,
                                         bias=kb_sb[:, jt:jt + 1], scale=SCALE)
                    if mrel < 4:
                        nc.vector.tensor_tensor(
                            pt[:, ia:ib], pt[:, ia:ib],
                            tri_sb[:, mrel * 512 + ia:mrel * 512 + ib], OP.mult)
                    elif mrel >= 8:
                        nc.vector.tensor_tensor(
                            pt[:, ia:ib], pt[:, ia:ib],
                            tri_sb[:, (mrel - 4) * 512 + ia:(mrel - 4) * 512 + ib],
                            OP.mult)
                    first, last = (mi == 0), (mrel == 11)
                    nc.tensor.matmul(dn_ps[:, ia:ib], onesbf_sb[:], pt[:, ia:ib],
                                     start=first, stop=last, skip_group_check=True)
                    for dh in range(2):
                        nc.tensor.matmul(y_ps[dh][:, ia:ib],
                                         vtc[:, jh, dh * 128:(dh + 1) * 128],
                                         pt[:, ia:ib], start=first, stop=last,
                                         skip_group_check=True)
                rc = spool.tile([128, 512], f32, tag="rc")
                nc.vector.reciprocal_approx_fast(rc[:], dn_ps[:])
                for dh in range(2):
                    nc.vector.tensor_tensor(yt_sb[:, 2 * h + dh, :],
                                            y_ps[dh][:], rc[:], OP.mult)
            return yt_sb

        def o_proj(a, yt_sb):
            for msub in range(4):
                for dmh in range(2):
                    o_sb = opool.tile([128, 1024], f16, tag="o")
                    for dq in range(2):
                        c0 = (dmh * 2 + dq) * 512
                        o_ps = pp_small.tile([128, 512], f32, tag="psm")
                        for hd in range(4):
                            nc.tensor.matmul(o_ps[:],
                                             yt_sb[:, hd, msub * 128:(msub + 1) * 128],
                                             wo_sb[:, hd, c0:c0 + 512],
                                             start=(hd == 0), stop=(hd == 3))
                        nc.scalar.copy(o_sb[:, dq * 512:(dq + 1) * 512], o_ps[:])
                    r0_ = a * 512 + msub * 128
                    nc.sync.dma_start(o_d.ap()[r0_:r0_ + 128, dmh * 1024:(dmh + 1) * 1024],
                                      o_sb[:])

        # ================= schedule =================
        kv_proj(4)
        kv_proj(5)

        if HALO_CC:
            # export my tiles 4,5 (tokens [1024,2048) of my half) to the pair
            nc.gpsimd.dma_start(hx_in.ap()[0], kt_tiles[4][:])
            nc.gpsimd.dma_start(hx_in.ap()[1], kt_tiles[5][:])
            nc.gpsimd.dma_start(hx_in.ap()[2], vt_tiles[4][:])
            nc.gpsimd.dma_start(hx_in.ap()[3], vt_tiles[5][:])
            nc.gpsimd.collective_compute(
                "AllGather",
                mybir.AluOpType.bypass,
                replica_groups=[[0, 1], [2, 3], [4, 5], [6, 7]],
                ins=[hx_in.ap().opt()],
                outs=[hx_out.ap().opt()],
            )

        nc.sync.dma_start(wq_sb[:], wq_d.ap().rearrange("(ko p) c -> p ko c", p=128))
        load_x(2)
        kv_proj(2)
        nc.sync.dma_start(tri_sb[:], tri_d.ap())
        nc.sync.dma_start(wo_sb[:], wo_d.ap().rearrange("(hd p) c -> p hd c", p=128))
        load_x(3)
        kv_proj(3)

        if HALO_CC:
            # import pair slot 0 (= core (g,0)'s tiles) as my tiles 0,1.
            # For (g,0) this is self-data, fully masked by kbias.
            kt0 = kpool.tile([128, 2, 512], bf16, tag="kt")
            nc.gpsimd.dma_start(kt0[:], hx_out.ap()[0, 0])
            kt1 = kpool.tile([128, 2, 512], bf16, tag="kt")
            nc.gpsimd.dma_start(kt1[:], hx_out.ap()[0, 1])
            vt0 = vpool.tile([128, 4, 256], bf16, tag="vt")
            nc.gpsimd.dma_start(vt0[:], hx_out.ap()[0, 2])
            vt1 = vpool.tile([128, 4, 256], bf16, tag="vt")
            nc.gpsimd.dma_start(vt1[:], hx_out.ap()[0, 3])
            kt_tiles[0], kt_tiles[1] = kt0, kt1
            vt_tiles[0], vt_tiles[1] = vt0, vt1
        else:
            load_x(0)
            kv_proj(0)
            load_x(1)
            kv_proj(1)

        yt_prev = {}
        border = [2, 3, 0, 1]
        xq = {2: load_x_halves(4)}   # queries of block a live in x tile a+2
        for i, a in enumerate(border):
            if i + 1 < 4:
                nxt = border[i + 1]
                xq[nxt] = load_x_halves(nxt + 2)   # prefetch next block's x
            qt = q_proj(a, xq.pop(a))
            if i > 0:
                pa = border[i - 1]
                o_proj(pa, yt_prev.pop(pa))
            yt_prev[a] = attention(a, qt)
        o_proj(1, yt_prev[1])

    nc.compile()
    _cache["nc"] = nc
    return nc


def _run(inputs, trace=False):
    from concourse.bass_utils import run_bass_kernel_spmd

    nc = _build_program()
    in_maps = _host_prep(**inputs)
    res = run_bass_kernel_spmd(nc, in_maps, core_ids=list(range(8)), trace=trace)
    full = np.zeros((T, DM), np.float32)
    for g in range(NG):
        for s in range(NS):
            full[s * 2048:(s + 1) * 2048] += res.results[g * 2 + s]["o_part"].astype(np.float32)
    return full.reshape(1, T, DM), res


def kernel(**inputs):
    return _run(inputs, trace=False)[0]
